# revision 38
# baseline (speedup 1.0000x reference)
"""Trainium2 Bass kernel for a 6-layer caption-generator transformer.

Sharding: data-parallel over batch (16 -> 2 per core) for the 6 transformer
layers; vocab-sharded final projection + softmax (50257 -> 6283 cols/core)
with an AllGather of final hidden states and pipelined grouped AllReduces
of the softmax denominators.

Self-contained: hardcodes all shapes; builds + compiles the Bass/Tile
program on first call (cached) and runs it on 8 NeuronCores via
run_bass_kernel_spmd.
"""

import numpy as np
import ml_dtypes
from contextlib import ExitStack

# ---- model dims (hardcoded from the problem spec) ----
B, IMG, TXT = 16, 197, 24
S = IMG + TXT + 1          # 222
D, H, L, V = 768, 12, 6, 50257
HD = D // H                # 64
F = 4 * D                  # 3072
EPS = 1e-5
NCORE = 8
BC = B // NCORE            # 2 batches per core
NT = BC * S                # 444 tokens per core
KD = D // 128              # 6 k-tiles over D
KF = F // 128              # 24 k-tiles over F
ATT_SCALE = 1.0 / float(np.sqrt(np.float32(HD)))

# local (per-batch) token tiles: (offset, len)
LT = [(0, 128), (128, S - 128)]          # [(0,128),(128,94)]
# per-core token tiles within the 444-token block
MT_LOC = [(0, 128), (128, 128), (256, 128), (384, NT - 384)]

# vocab sharding
VC = 6283                   # vocab cols per core (8*6283 = 50264 >= 50257)
V_PAD = VC * NCORE          # 50264
N_VPAD = V_PAD - V          # 7 padded cols (wout=0 -> logits 0 -> exp 1)
VCH = [(i * 512, 512) for i in range(12)] + [(6144, VC - 6144)]   # 12x512+139

N_MT = NCORE * len(MT_LOC)  # 32 token m-tiles over the gathered 3552 tokens
ZGRP = 4                    # m-tiles per Z AllReduce group
N_ZG = N_MT // ZGRP         # 8 groups

_CACHE = {}
PIPE_SKEW = True


def _build(flags, repeat=1, sim_mode=False, n_layers=L, do_final=True,
           ablate=()):
    import concourse.bass as bass
    import concourse.tile as tile
    import concourse.mybir as mybir
    from concourse import bacc

    f32 = mybir.dt.float32
    f32r = mybir.dt.float32r
    bf16 = mybir.dt.bfloat16
    AF = mybir.ActivationFunctionType
    OP = mybir.AluOpType
    AX = mybir.AxisListType

    bv_nz, b2_nz, bout_nz, ln_ident = flags

    nc = bacc.Bacc("TRN2", target_bir_lowering=False, debug=False,
                   num_devices=NCORE)

    # ---- DRAM I/O ----
    d_x0 = nc.dram_tensor("x0t", [D, NT], bf16, kind="ExternalInput").ap()
    d_wqk = nc.dram_tensor("wqk", [L, D, 2 * D], bf16, kind="ExternalInput").ap()
    d_wv = nc.dram_tensor("wv", [L, D, D], bf16, kind="ExternalInput").ap()
    d_w1 = nc.dram_tensor("w1", [L, D, F], bf16, kind="ExternalInput").ap()
    d_w2 = nc.dram_tensor("w2", [L, F, D], bf16, kind="ExternalInput").ap()
    d_wo = nc.dram_tensor("wout", [D, VC], bf16, kind="ExternalInput").ap()
    d_pp = nc.dram_tensor("pp", [L, 128, 72], f32, kind="ExternalInput").ap()
    d_mask = nc.dram_tensor("maskt", [BC, S, S], bf16, kind="ExternalInput").ap()
    d_ones = nc.dram_tensor("ones", [128, 128], f32r, kind="ExternalInput").ap()
    d_ident = nc.dram_tensor("ident", [128, 128], f32r, kind="ExternalInput").ap()
    d_bvbc = (nc.dram_tensor("bvbc", [L, 128, D], f32, kind="ExternalInput").ap()
              if bv_nz else None)
    d_bout = (nc.dram_tensor("boutbc", [128, VC], f32, kind="ExternalInput").ap()
              if bout_nz else None)
    d_out = nc.dram_tensor("out", [NCORE * NT, VC], bf16,
                           kind="ExternalOutput").ap()

    with tile.TileContext(nc) as tc, ExitStack() as top, \
            nc.allow_low_precision(reason="fp32r matmul tiles"):
        # psum pools shared across phases via tags (4+4 banks)
        pps = top.enter_context(tc.tile_pool(name="pps", bufs=5, space="PSUM"))
        ppa = top.enter_context(tc.tile_pool(name="ppa", bufs=3, space="PSUM"))
        pdram = top.enter_context(tc.tile_pool(name="pdram", bufs=1, space="DRAM"))
        pconst = top.enter_context(tc.tile_pool(name="pconst", bufs=1))

        ones_sb = pconst.tile([128, 128], f32r, tag="ones")
        nc.sync.dma_start(ones_sb[:], d_ones[:])
        ident_sb = pconst.tile([128, 128], f32r, tag="ident")
        nc.sync.dma_start(ident_sb[:], d_ident[:])
        eps_sb = pconst.tile([128, 1], f32, tag="eps")
        nc.vector.memset(eps_sb[:], EPS)
        nones_sb = pconst.tile([1, 128], f32r, tag="nones")
        nc.vector.memset(nones_sb[:, :].bitcast(mybir.dt.uint32), 0xBF800000)
        onesb_sb = pconst.tile([128, 1], bf16, tag="onesb")
        nc.vector.memset(onesb_sb[:], 1.0)
        identb_sb = pconst.tile([128, 128], bf16, tag="identb")
        nc.vector.tensor_copy(identb_sb[:], ident_sb[:])

        def mmr(out, lhsT, rhs, **kw):
            nc.tensor.matmul(out, lhsT, rhs, **kw)

        def _emit(rep):
            # dram staging for collectives
            ag_in = pdram.tile([D, NT], bf16, name=f"agin{rep}")
            ag_out = pdram.tile([NCORE, D, NT], bf16,
                                addr_space="Local" if sim_mode else "Shared",
                                name=f"agout{rep}")
            z_in = [pdram.tile([128, ZGRP], f32, name=f"zin{rep}_{g}")
                    for g in range(N_ZG)]
            z_out = [pdram.tile([128, ZGRP], f32, name=f"zout{rep}_{g}",
                                addr_space="Local" if sim_mode else "Shared")
                     for g in range(N_ZG)]

            with ExitStack() as lay:
                px = lay.enter_context(tc.tile_pool(name=f"px{rep}", bufs=32))
                pqk = lay.enter_context(tc.tile_pool(name=f"pqk{rep}", bufs=26))
                pv = lay.enter_context(tc.tile_pool(name=f"pv{rep}", bufs=5))
                patt = lay.enter_context(tc.tile_pool(name=f"patt{rep}", bufs=8))
                pescr = lay.enter_context(tc.tile_pool(name=f"pescr{rep}", bufs=4))
                pocat = lay.enter_context(tc.tile_pool(name=f"pocat{rep}", bufs=5))
                ph = lay.enter_context(tc.tile_pool(name=f"ph{rep}", bufs=50))
                plns = lay.enter_context(tc.tile_pool(name=f"plns{rep}", bufs=6))
                pmask = lay.enter_context(tc.tile_pool(name=f"pmask{rep}", bufs=4))
                pwmed = lay.enter_context(tc.tile_pool(name=f"pwmed{rep}", bufs=12))
                pwsm = lay.enter_context(tc.tile_pool(name=f"pwsm{rep}", bufs=18))
                pw2 = lay.enter_context(tc.tile_pool(name=f"pw2{rep}", bufs=50))
                ppp = lay.enter_context(tc.tile_pool(name=f"ppp{rep}", bufs=2))
                pst = lay.enter_context(tc.tile_pool(name=f"pst{rep}", bufs=16))
                pbv = (lay.enter_context(tc.tile_pool(name=f"pbv{rep}", bufs=2))
                       if bv_nz else None)

                # initial x (transposed, per-batch [D, S] bf16)
                xt = {b: [] for b in range(BC)}
                for b in range(BC):
                    for k in range(KD):
                        t = px.tile([128, S], bf16, tag="x")
                        nc.sync.dma_start(t[:], d_x0[k * 128:(k + 1) * 128,
                                                     b * S:(b + 1) * S])
                        xt[b].append(t)

                # mask tiles (resident; maskT[b][t,s] layout)
                mk = {}
                for b in range(BC):
                    for ti, (t0, tl) in enumerate(LT):
                        m = pmask.tile([128, S], bf16, tag="mask")
                        nc.sync.dma_start(m[:tl, :], d_mask[b, t0:t0 + tl, :])
                        mk[(b, ti)] = m

                state = {"pending": None}

                def qkproj(b, wq_sb, wk_sb, pp_sb):
                    qk = []
                    for m in range(12):
                        w = wq_sb if m < 6 else wk_sb
                        mm = m % 6
                        ps = pps.tile([128, S], f32, tag="mm")
                        for k in range(KD):
                            mmr(ps[:], w[k][:, mm * 128:(mm + 1) * 128],
                                xt[b][k][:], start=(k == 0), stop=(k == KD - 1))
                        sb = pqk.tile([128, S], bf16, tag="qk")
                        nc.scalar.activation(sb[:], ps[:], AF.Identity,
                                             bias=pp_sb[:, 30 + m:31 + m])
                        qk.append(sb)
                    return qk

                def vproj(b, wv_sb, bv_sb):
                    vs = {}
                    for ti, (t0, tl) in enumerate(LT):
                        vt = pv.tile([128, 12 * 65], bf16, tag="v")
                        vt3 = vt[:, :].rearrange("p (h e) -> p h e", e=65)
                        nc.vector.memset(vt3[:tl, :, 64:65], 1.0)
                        for n in range(2):
                            ps = ppa.tile([128, 384], f32, tag="aux")
                            for k in range(KD):
                                mmr(ps[:tl, :], xt[b][k][:, t0:t0 + tl],
                                    wv_sb[k][:, n * 384:(n + 1) * 384],
                                    start=(k == 0), stop=(k == KD - 1))
                            if bv_nz:
                                nc.vector.tensor_add(
                                    ps[:tl, :], ps[:tl, :],
                                    bv_sb[:tl, n * 384:(n + 1) * 384])
                            ps3 = ps[:, :].rearrange("p (h e) -> p h e", e=64)
                            nc.scalar.activation(
                                vt3[:tl, n * 6:(n + 1) * 6, 0:64],
                                ps3[:tl, :, :], AF.Identity)
                        vs[ti] = vt
                    return vs

                def attn(b, qk, vs):
                    oc = {}
                    for st in range(2):
                        oc[st] = pocat.tile([128, D], bf16, tag="ocat",
                                            name="ocat")
                    for half in range(2):
                        atts = {}
                        for hh in range(6):
                            h = half * 6 + hh
                            hq = qk[h // 2]
                            hk = qk[6 + h // 2]
                            hb = (h % 2) * 64
                            att = {}
                            for ti, (t0, tl) in enumerate(LT):
                                ps = pps.tile([128, S], f32, tag="mm")
                                nc.tensor.matmul(
                                    ps[:tl, t0:S],
                                    hk[hb:hb + 64, t0:t0 + tl],
                                    hq[hb:hb + 64, t0:S],
                                    start=True, stop=True)
                                esc = pescr.tile([128, S], f32, tag="escr")
                                nc.scalar.activation(
                                    esc[:tl, t0:S], ps[:tl, t0:S],
                                    AF.Exp, scale=ATT_SCALE)
                                at = patt.tile([128, S], bf16, tag="att")
                                nc.gpsimd.tensor_mul(
                                    at[:tl, t0:S], esc[:tl, t0:S],
                                    mk[(b, ti)][:tl, t0:S])
                                att[ti] = at
                            atts[hh] = att
                        for st in range(2):
                            s0, sl = LT[st]
                            tis = [ti for ti, (t0, tl) in enumerate(LT)
                                   if t0 < s0 + sl]
                            po6 = ppa.tile([128, 390], f32, tag="aux",
                                           name="po6")
                            for hh in range(6):
                                h = half * 6 + hh
                                for i, ti in enumerate(tis):
                                    t0, tl = LT[ti]
                                    nc.tensor.matmul(
                                        po6[:sl, hh * 65:(hh + 1) * 65],
                                        atts[hh][ti][:tl, s0:s0 + sl],
                                        vs[ti][:tl, h * 65:(h + 1) * 65],
                                        start=(i == 0),
                                        stop=(i == len(tis) - 1))
                            po63 = po6[:, :].rearrange("p (h e) -> p h e", e=65)
                            rz6 = pst.tile([128, 6], f32, tag="row")
                            nc.vector.reciprocal(rz6[:sl, :],
                                                 po63[:sl, :, 64:65])
                            for hh in range(6):
                                h = half * 6 + hh
                                if hh % 2 == 0:
                                    nc.vector.tensor_scalar_mul(
                                        oc[st][:sl, h * 64:(h + 1) * 64],
                                        po63[:sl, hh, 0:64],
                                        rz6[:sl, hh:hh + 1])
                                else:
                                    nc.scalar.activation(
                                        oc[st][:sl, h * 64:(h + 1) * 64],
                                        po63[:sl, hh, 0:64], AF.Identity,
                                        scale=rz6[:sl, hh:hh + 1])
                    return oc

                def resid(b, oc):
                    """transpose o + residual add -> x2t[b]"""
                    x2 = []
                    for k in range(KD):
                        x2.append(px.tile([128, S], bf16, tag="x", name="x2t"))
                    for st in range(2):
                        s0, sl = LT[st]
                        for k in range(KD):
                            pt = ppa.tile([128, 128], bf16, tag="aux")
                            nc.tensor.transpose(
                                pt[:, :sl],
                                oc[st][:sl, k * 128:(k + 1) * 128],
                                identb_sb[:sl, :sl])
                            nc.vector.tensor_add(x2[k][:, s0:s0 + sl],
                                                 xt[b][k][:, s0:s0 + sl],
                                                 pt[:, :sl])
                    return x2

                def ln_stats(src):
                    """col-sums of x and x^2 -> (mu, var+eps) SBUF rows.

                    Frees the stat PSUM rows immediately (they must not stay
                    live across the interleaved attention block)."""
                    ps_mu = pps.tile([1, 256], f32, tag="mm")
                    for k in range(KD):
                        mmr(ps_mu[:, :S], onesb_sb[:, 0:1], src[k][:],
                            start=(k == 0), stop=(k == KD - 1))
                    mu = pst.tile([1, 256], f32, tag="row")
                    nc.vector.tensor_scalar_mul(mu[:, :S], ps_mu[:, :S], 1.0 / D)
                    ps_sq = pps.tile([1, 256], f32, tag="mm")
                    for k in range(KD):
                        s = plns.tile([128, S], bf16, tag="lnscr")
                        nc.gpsimd.tensor_mul(s[:], src[k][:], src[k][:])
                        mmr(ps_sq[:, :S], onesb_sb[:, 0:1], s[:],
                            start=(k == 0), stop=(k == KD - 1))
                    var = pst.tile([1, 256], f32, tag="row")
                    nc.vector.tensor_scalar(var[:, :S], ps_sq[:, :S], 1.0 / D,
                                            EPS, OP.mult, OP.add)
                    return mu, var

                def ln_rows(stats):
                    """serial rsqrt/mean row chain on DVE+Pool -> (rinv, cc).

                    Emit EARLY (right after ln_stats) so the chain overlaps
                    the other batch's PE work; no act tables involved."""
                    mu, var = stats
                    musq = pst.tile([1, 256], f32, tag="row")
                    nc.gpsimd.tensor_mul(musq[:, :S], mu[:, :S], mu[:, :S])
                    nc.vector.tensor_sub(var[:, :S], var[:, :S], musq[:, :S])
                    # rsqrt(var) = Newton from seed 1/((1+v)/2) ~= v^-1/2
                    # near v~1 (LN variances cluster there); float ops only.
                    sh2 = pst.tile([1, 256], f32, tag="row")
                    nc.vector.tensor_scalar(sh2[:, :S], var[:, :S], 0.5, 0.5,
                                            OP.mult, OP.add)
                    seed = pst.tile([1, 256], f32, tag="row")
                    nc.vector.reciprocal(seed[:, :S], sh2[:, :S])
                    hv = pst.tile([1, 256], f32, tag="row")
                    nc.gpsimd.tensor_scalar_mul(hv[:, :S], var[:, :S], -0.5)
                    t2 = pst.tile([1, 256], f32, tag="row")
                    s2 = pst.tile([1, 256], f32, tag="row")
                    # NR1 (f32 scratch)
                    nc.vector.tensor_mul(t2[:, :S], seed[:, :S], seed[:, :S])
                    nc.vector.tensor_mul(t2[:, :S], t2[:, :S], hv[:, :S])
                    nc.vector.tensor_scalar_add(t2[:, :S], t2[:, :S], 1.5)
                    nc.vector.tensor_mul(s2[:, :S], seed[:, :S], t2[:, :S])
                    # NR2 (final product lands rounded in the f32r tile)
                    rinv = pst.tile([1, 256], f32r, tag="row")
                    nc.vector.memset(rinv[:, S:].bitcast(mybir.dt.uint32),
                                     0x3F800000)
                    nc.vector.tensor_mul(t2[:, :S], s2[:, :S], s2[:, :S])
                    nc.vector.tensor_mul(t2[:, :S], t2[:, :S], hv[:, :S])
                    nc.vector.tensor_scalar_add(t2[:, :S], t2[:, :S], 1.5)
                    nc.vector.tensor_mul(rinv[:, :S], s2[:, :S], t2[:, :S])
                    cc = pst.tile([1, 256], f32r, tag="row")
                    nc.gpsimd.memset(cc[:, S:].bitcast(mybir.dt.uint32), 0)
                    nc.gpsimd.tensor_mul(cc[:, :S], mu[:, :S], rinv[:, :S])
                    return rinv, cc

                def ln_apply(rows, src, s_col, b_col, pp_sb):
                    rinv, cc = rows
                    pa = pps.tile([128, 256], f32, tag="mm")
                    mmr(pa[:], ones_sb[0:1, :], rinv[:], start=True, stop=True)
                    pc = pps.tile([128, 256], f32, tag="mm")
                    mmr(pc[:], nones_sb[:, :], cc[:], start=True, stop=True)
                    out = []
                    for k in range(KD):
                        y = px.tile([128, S], bf16, tag="x")
                        if ln_ident:
                            nc.vector.tensor_mul(y[:], src[k][:], pa[:, :S])
                            nc.vector.tensor_add(y[:], y[:], pc[:, :S])
                        else:
                            t1 = plns.tile([128, S], f32, tag="lnscr2")
                            nc.vector.tensor_mul(t1[:], src[k][:], pa[:, :S])
                            nc.vector.tensor_add(t1[:], t1[:], pc[:, :S])
                            nc.vector.tensor_scalar(
                                y[:], t1[:],
                                pp_sb[:, s_col + k:s_col + k + 1],
                                pp_sb[:, b_col + k:b_col + k + 1],
                                OP.mult, OP.add)
                        out.append(y)
                    return out

                def ffn1(b, half, w1h, y1, pp_sb):
                    hs = []
                    for m in range(12):
                        fm = half * 12 + m
                        ps = pps.tile([128, S], f32, tag="mm")
                        for k in range(KD):
                            mmr(ps[:], w1h[k][:, m * 128:(m + 1) * 128],
                                y1[k][:], start=(k == 0), stop=(k == KD - 1))
                        hb16 = ph.tile([128, S], bf16, tag="h")
                        nc.scalar.activation(hb16[:], ps[:], AF.Relu,
                                             bias=pp_sb[:, 42 + fm:43 + fm])
                        hs.append(hb16)
                    return hs

                def ffn2(b, half, w2h, hts, y1, x3, pp_sb):
                    for m in range(3):
                        dm = half * 3 + m
                        ps = pps.tile([128, S], f32, tag="mm")
                        for k in range(KF):
                            nc.tensor.matmul(ps[:],
                                             w2h[k][:, m * 128:(m + 1) * 128],
                                             hts[k][:], start=(k == 0),
                                             stop=(k == KF - 1))
                        x3k = px.tile([128, S], bf16, tag="x")
                        nc.vector.tensor_add(x3k[:], ps[:], y1[dm][:])
                        if b2_nz:
                            nc.vector.tensor_scalar_add(
                                x3k[:], x3k[:], pp_sb[:, 24 + dm:25 + dm])
                        x3.append(x3k)

                for l in range(n_layers):
                    pp_sb = ppp.tile([128, 72], f32, tag="pp")
                    nc.sync.dma_start(pp_sb[:], d_pp[l])

                    wq_sb, wk_sb, wv_sb = [], [], []
                    for k in range(KD):
                        t = pwsm.tile([128, D], bf16, tag="wsm")
                        nc.sync.dma_start(t[:], d_wqk[l, k * 128:(k + 1) * 128,
                                                      0:D])
                        wq_sb.append(t)
                    for k in range(KD):
                        t = pwsm.tile([128, D], bf16, tag="wsm")
                        nc.sync.dma_start(t[:], d_wqk[l, k * 128:(k + 1) * 128,
                                                      D:2 * D])
                        wk_sb.append(t)
                    for k in range(KD):
                        t = pwsm.tile([128, D], bf16, tag="wsm")
                        nc.sync.dma_start(t[:], d_wv[l, k * 128:(k + 1) * 128, :])
                        wv_sb.append(t)
                    bv_sb = None
                    if bv_nz:
                        bv_sb = pbv.tile([128, D], f32, tag="bv")
                        nc.sync.dma_start(bv_sb[:], d_bvbc[l])
                    w1h = {}
                    for half in range(2):
                        w1h[half] = []
                        for k in range(KD):
                            t = pwmed.tile([128, F // 2], bf16, tag="wmed")
                            nc.sync.dma_start(
                                t[:], d_w1[l, k * 128:(k + 1) * 128,
                                           half * (F // 2):
                                           (half + 1) * (F // 2)])
                            w1h[half].append(t)
                    w2h = {}
                    for half in range(2):
                        w2h[half] = []
                        for k in range(KF):
                            t = pw2.tile([128, 384], bf16, tag="w2")
                            nc.sync.dma_start(
                                t[:], d_w2[l, k * 128:(k + 1) * 128,
                                           half * 384:(half + 1) * 384])
                            w2h[half].append(t)

                    # batch-0 QKV projections (xt[0] ready)
                    qk0 = qkproj(0, wq_sb, wk_sb, pp_sb)
                    vs0 = vproj(0, wv_sb, bv_sb)
                    # finish previous layer's LN2 for batch 1 (overlaps above)
                    if state["pending"] is not None:
                        xt[1] = state["pending"]()
                        state["pending"] = None
                    oc0 = attn(0, qk0, vs0)
                    qk1 = qkproj(1, wq_sb, wk_sb, pp_sb)
                    vs1 = vproj(1, wv_sb, bv_sb)
                    x2_0 = resid(0, oc0)
                    st1_0 = ln_stats(x2_0)
                    r1_0 = ln_rows(st1_0)
                    oc1 = attn(1, qk1, vs1)
                    y1_0 = ln_apply(r1_0, x2_0, 0, 6, pp_sb)
                    x2_1 = resid(1, oc1)
                    st1_1 = ln_stats(x2_1)
                    r1_1 = ln_rows(st1_1)

                    ht0 = ffn1(0, 0, w1h[0], y1_0, pp_sb)
                    ht0 += ffn1(0, 1, w1h[1], y1_0, pp_sb)
                    y1_1 = ln_apply(r1_1, x2_1, 0, 6, pp_sb)
                    ht1 = ffn1(1, 0, w1h[0], y1_1, pp_sb)
                    ht1 += ffn1(1, 1, w1h[1], y1_1, pp_sb)

                    x3_0, x3_1 = [], []
                    ffn2(0, 0, w2h[0], ht0, y1_0, x3_0, pp_sb)
                    ffn2(0, 1, w2h[1], ht0, y1_0, x3_0, pp_sb)
                    st2_0 = ln_stats(x3_0)
                    r2_0 = ln_rows(st2_0)
                    ffn2(1, 0, w2h[0], ht1, y1_1, x3_1, pp_sb)
                    xt[0] = ln_apply(r2_0, x3_0, 12, 18, pp_sb)
                    ffn2(1, 1, w2h[1], ht1, y1_1, x3_1, pp_sb)
                    st2_1 = ln_stats(x3_1)
                    r2_1 = ln_rows(st2_1)
                    if PIPE_SKEW:
                        state["pending"] = (
                            lambda r=r2_1, x3=x3_1, pp=pp_sb:
                            ln_apply(r, x3, 12, 18, pp))
                    else:
                        xt[1] = ln_apply(r2_1, x3_1, 12, 18, pp_sb)

                if state["pending"] is not None:
                    xt[1] = state["pending"]()
                    state["pending"] = None

                # ship final x (bf16) straight to DRAM for the AllGather
                for b in range(BC):
                    for k in range(KD):
                        nc.sync.dma_start(
                            ag_in[k * 128:(k + 1) * 128, b * S:(b + 1) * S],
                            xt[b][k][:])

            # ================= final: AllGather + vocab-sharded projection ======
            if not do_final:
                t_dump = pconst.tile([128, 8], bf16, tag="dump", name=f"dump{rep}")
                nc.sync.dma_start(t_dump[:], ag_in[0:128, 0:8])
                nc.sync.dma_start(d_out[0:128, 0:8], t_dump[:])
                return
            with ExitStack() as fin:
                if sim_mode:
                    for c in range(NCORE):
                        nc.sync.dma_start(ag_out[c], ag_in[:])
                else:
                    nc.gpsimd.collective_compute(
                        "AllGather", mybir.AluOpType.bypass,
                        replica_groups=[list(range(NCORE))],
                        ins=[ag_in.opt()], outs=[ag_out.opt()])

                pwo = fin.enter_context(tc.tile_pool(name=f"pwo{rep}", bufs=6))
                pxa = fin.enter_context(tc.tile_pool(name=f"pxa{rep}", bufs=12))
                pstrip = fin.enter_context(tc.tile_pool(name=f"pstrip{rep}", bufs=8))
                pstg = fin.enter_context(tc.tile_pool(name=f"pstg{rep}", bufs=8))
                pz = fin.enter_context(tc.tile_pool(name=f"pz{rep}", bufs=N_MT + 2))
                pzr = fin.enter_context(tc.tile_pool(name=f"pzr{rep}", bufs=6))
                pbo = (fin.enter_context(tc.tile_pool(name=f"pbo{rep}", bufs=1))
                       if bout_nz else None)

                wo_sb = []
                for k in range(KD):
                    t = pwo.tile([128, VC], bf16, tag="wo")
                    nc.sync.dma_start(t[:], d_wo[k * 128:(k + 1) * 128, :])
                    wo_sb.append(t)
                if bout_nz:
                    bo_sb = pbo.tile([128, VC], f32, tag="bo")
                    nc.sync.dma_start(bo_sb[:], d_bout[:])

                zg_sb = pzr.tile([128, N_MT], f32, tag="zg")
                nc.vector.memset(zg_sb[:], 1.0)
                rz_sb = pzr.tile([128, N_MT], f32, tag="rzf")

                strips = {}
                zp = {}
                xa_c = {}
                for mt in range(N_MT):
                    c, j = mt // 4, mt % 4
                    m0, ml = MT_LOC[j]
                    if c not in xa_c:
                        ts = []
                        for k in range(KD):
                            t = pxa.tile([128, NT], bf16, tag="xa")
                            nc.sync.dma_start(t[:],
                                              ag_out[c, k * 128:(k + 1) * 128, :])
                            ts.append(t)
                        xa_c[c] = ts
                    xa = xa_c[c]
                    strip = pstrip.tile([128, VC], bf16, tag="strip")
                    strips[mt] = strip
                    zpt = pz.tile([128, len(VCH)], f32, tag="zp")
                    zp[mt] = zpt
                    for vi, (v0, vw) in enumerate(VCH):
                        ps = ppa.tile([128, 512], f32, tag="aux")
                        for k in range(KD):
                            nc.tensor.matmul(ps[:ml, :vw],
                                             xa[k][:, m0:m0 + ml],
                                             wo_sb[k][:, v0:v0 + vw],
                                             start=(k == 0), stop=(k == KD - 1))
                        if bout_nz:
                            nc.vector.tensor_add(ps[:ml, :vw], ps[:ml, :vw],
                                                 bo_sb[:ml, v0:v0 + vw])
                        nc.scalar.activation(strip[:ml, v0:v0 + vw], ps[:ml, :vw],
                                             AF.Exp,
                                             accum_out=zpt[:ml, vi:vi + 1])
                    nc.vector.tensor_reduce(zg_sb[:ml, mt:mt + 1], zpt[:ml, :],
                                            AX.X, OP.add)

                    if mt % ZGRP == ZGRP - 1:
                        g = mt // ZGRP
                        nc.sync.dma_start(z_in[g][:],
                                          zg_sb[:, g * ZGRP:(g + 1) * ZGRP])
                        if sim_mode:
                            nc.sync.dma_start(z_out[g][:], z_in[g][:])
                        else:
                            nc.gpsimd.collective_compute(
                                "AllReduce", mybir.AluOpType.add,
                                replica_groups=[list(range(NCORE))],
                                ins=[z_in[g].opt()], outs=[z_out[g].opt()])
                        zr = pzr.tile([128, ZGRP], f32, tag="zred")
                        nc.sync.dma_start(zr[:], z_out[g][:])
                        nc.vector.tensor_scalar_add(zr[:], zr[:], -float(N_VPAD))
                        nc.vector.reciprocal(
                            rz_sb[:, g * ZGRP:(g + 1) * ZGRP], zr[:])
                        for mt2 in range(g * ZGRP, (g + 1) * ZGRP):
                            c2, j2 = mt2 // 4, mt2 % 4
                            m02, ml2 = MT_LOC[j2]
                            r0 = c2 * NT + m02
                            for ci, (v0, vw) in enumerate(VCH):
                                so = pstg.tile([128, 512], bf16, tag="stg")
                                eng = nc.vector if ci % 2 == 0 else nc.gpsimd
                                eng.tensor_scalar_mul(
                                    so[:ml2, :vw],
                                    strips[mt2][:ml2, v0:v0 + vw],
                                    rz_sb[:ml2, mt2:mt2 + 1])
                                nc.sync.dma_start(d_out[r0:r0 + ml2, v0:v0 + vw],
                                                  so[:ml2, :vw])
                            del strips[mt2]

        for rep in range(repeat):
            _emit(rep)

    nc.compile()
    return nc


def _get_nc(flags, repeat=1, sim_mode=False, n_layers=L, do_final=True,
            ablate=()):
    key = (flags, repeat, sim_mode, n_layers, do_final, tuple(ablate))
    if key not in _CACHE:
        _CACHE[key] = _build(flags, repeat, sim_mode, n_layers, do_final,
                             ablate)
    return _CACHE[key]


def _prep(inputs):
    """Host-side preprocessing -> (per-core in_maps, specialization flags)."""
    x_img = np.asarray(inputs["image_token"], np.float32)
    tok = np.asarray(inputs["text_token"])
    tmask = np.asarray(inputs["text_mask"])
    temb = np.asarray(inputs["text_emb"], np.float32)
    semb = np.asarray(inputs["sep_emb"], np.float32)
    Wq = np.asarray(inputs["Wq"], np.float32)
    bq = np.asarray(inputs["bq"], np.float32)
    Wk = np.asarray(inputs["Wk"], np.float32)
    bk = np.asarray(inputs["bk"], np.float32)
    Wv = np.asarray(inputs["Wv"], np.float32)
    bv = np.asarray(inputs["bv"], np.float32)
    ln1_s = np.asarray(inputs["ln1_s"], np.float32)
    ln1_b = np.asarray(inputs["ln1_b"], np.float32)
    W1 = np.asarray(inputs["W1"], np.float32)
    b1 = np.asarray(inputs["b1"], np.float32)
    W2 = np.asarray(inputs["W2"], np.float32)
    b2 = np.asarray(inputs["b2"], np.float32)
    ln2_s = np.asarray(inputs["ln2_s"], np.float32)
    ln2_b = np.asarray(inputs["ln2_b"], np.float32)
    Wout = np.asarray(inputs["Wout"], np.float32)
    bout = np.asarray(inputs["bout"], np.float32)

    # x0 = [img | sep | emb[tokens]]
    x0 = np.concatenate(
        [x_img, np.broadcast_to(semb[None], (B, 1, D)), temb[tok]], axis=1)

    # maskT[b][t,s] = (t<=s) & combined[b,t]
    comb = np.concatenate(
        [np.ones((B, S - TXT), bool), tmask.astype(bool)], axis=1)
    tril_t = np.tril(np.ones((S, S), bool)).T  # [t,s]: t<=s
    maskt = (tril_t[None] & comb[:, :, None]).astype(ml_dtypes.bfloat16)

    # packed weights (bf16)
    wqk = np.ascontiguousarray(np.concatenate([
        Wq.transpose(0, 2, 1, 3).reshape(L, D, D),
        Wk.transpose(0, 2, 1, 3).reshape(L, D, D)],
        axis=2).astype(ml_dtypes.bfloat16))
    wv = np.ascontiguousarray(
        Wv.transpose(0, 2, 1, 3).reshape(L, D, D).astype(ml_dtypes.bfloat16))
    w1 = W1.astype(ml_dtypes.bfloat16)
    w2 = W2.astype(ml_dtypes.bfloat16)

    # per-partition params: [L,128,72]
    pp = np.zeros((L, 128, 72), np.float32)

    def put(dst0, arr):  # arr [L, n*128]
        n = arr.shape[1] // 128
        pp[:, :, dst0:dst0 + n] = arr.reshape(L, n, 128).transpose(0, 2, 1)

    put(0, ln1_s); put(6, ln1_b); put(12, ln2_s); put(18, ln2_b); put(24, b2)
    put(30, np.concatenate([bq.reshape(L, D), bk.reshape(L, D)], axis=1))
    put(42, b1)

    wo_pad = np.zeros((D, V_PAD), ml_dtypes.bfloat16)
    wo_pad[:, :V] = Wout.astype(ml_dtypes.bfloat16)
    bout_pad = np.zeros(V_PAD, np.float32)
    bout_pad[:V] = bout

    ln_ident = bool(np.all(ln1_s == 1.0) and np.all(ln1_b == 0.0)
                    and np.all(ln2_s == 1.0) and np.all(ln2_b == 0.0))
    flags = (bool(np.any(bv)), bool(np.any(b2)), bool(np.any(bout)), ln_ident)

    ones = np.ones((128, 128), np.float32)
    ident = np.eye(128, dtype=np.float32)

    in_maps = []
    for c in range(NCORE):
        m = {
            "x0t": np.ascontiguousarray(
                x0[c * BC:(c + 1) * BC].reshape(NT, D).T
                .astype(ml_dtypes.bfloat16)),
            "wqk": wqk, "wv": wv, "w1": w1, "w2": w2,
            "wout": np.ascontiguousarray(wo_pad[:, c * VC:(c + 1) * VC]),
            "pp": pp,
            "maskt": np.ascontiguousarray(maskt[c * BC:(c + 1) * BC]),
            "ones": ones, "ident": ident,
        }
        if flags[0]:
            m["bvbc"] = np.ascontiguousarray(np.broadcast_to(
                bv.reshape(L, 1, D), (L, 128, D)))
        if flags[2]:
            m["boutbc"] = np.ascontiguousarray(np.broadcast_to(
                bout_pad[c * VC:(c + 1) * VC][None], (128, VC)))
        in_maps.append(m)
    return in_maps, flags


def kernel(**inputs):
    from concourse.bass_utils import run_bass_kernel_spmd
    in_maps, flags = _prep(inputs)
    nc = _get_nc(flags)
    res = run_bass_kernel_spmd(nc, in_maps, list(range(NCORE)))
    full = np.concatenate([res.results[c]["out"] for c in range(NCORE)], axis=1)
    return np.ascontiguousarray(
        full[:, :V].astype(np.float32).reshape(B, S, V))



# revision 39
# speedup vs baseline: 1.0636x; 1.0636x over previous
"""Trainium2 Bass kernel for a 6-layer caption-generator transformer.

Sharding: data-parallel over batch (16 -> 2 per core) for the 6 transformer
layers; vocab-sharded final projection + softmax (50257 -> 6283 cols/core)
with an AllGather of final hidden states and pipelined grouped AllReduces
of the softmax denominators.

Self-contained: hardcodes all shapes; builds + compiles the Bass/Tile
program on first call (cached) and runs it on 8 NeuronCores via
run_bass_kernel_spmd.
"""

import numpy as np
import ml_dtypes
from contextlib import ExitStack

# ---- model dims (hardcoded from the problem spec) ----
B, IMG, TXT = 16, 197, 24
S = IMG + TXT + 1          # 222
D, H, L, V = 768, 12, 6, 50257
HD = D // H                # 64
F = 4 * D                  # 3072
EPS = 1e-5
NCORE = 8
BC = B // NCORE            # 2 batches per core
NT = BC * S                # 444 tokens per core
KD = D // 128              # 6 k-tiles over D
KF = F // 128              # 24 k-tiles over F
ATT_SCALE = 1.0 / float(np.sqrt(np.float32(HD)))

# local (per-batch) token tiles: (offset, len)
LT = [(0, 128), (128, S - 128)]          # [(0,128),(128,94)]
# per-core token tiles within the 444-token block
MT_LOC = [(0, 128), (128, 128), (256, 128), (384, NT - 384)]

# vocab sharding
VC = 6283                   # vocab cols per core (8*6283 = 50264 >= 50257)
V_PAD = VC * NCORE          # 50264
N_VPAD = V_PAD - V          # 7 padded cols (wout=0 -> logits 0 -> exp 1)
VCH = [(i * 512, 512) for i in range(12)] + [(6144, VC - 6144)]   # 12x512+139

N_MT = NCORE * len(MT_LOC)  # 32 token m-tiles over the gathered 3552 tokens
ZGRP = 4                    # m-tiles per Z AllReduce group
N_ZG = N_MT // ZGRP         # 8 groups

_CACHE = {}
PIPE_SKEW = True


def _build(flags, repeat=1, sim_mode=False, n_layers=L, do_final=True,
           ablate=()):
    import concourse.bass as bass
    import concourse.tile as tile
    import concourse.mybir as mybir
    from concourse import bacc

    f32 = mybir.dt.float32
    f32r = mybir.dt.float32r
    bf16 = mybir.dt.bfloat16
    AF = mybir.ActivationFunctionType
    OP = mybir.AluOpType
    AX = mybir.AxisListType

    bv_nz, b2_nz, bout_nz, ln_ident = flags

    nc = bacc.Bacc("TRN2", target_bir_lowering=False, debug=False,
                   num_devices=NCORE)

    # ---- DRAM I/O ----
    d_x0 = nc.dram_tensor("x0t", [D, NT], bf16, kind="ExternalInput").ap()
    d_wqk = nc.dram_tensor("wqk", [L, D, 2 * D], bf16, kind="ExternalInput").ap()
    d_wv = nc.dram_tensor("wv", [L, D, D], bf16, kind="ExternalInput").ap()
    d_w1 = nc.dram_tensor("w1", [L, D, F], bf16, kind="ExternalInput").ap()
    d_w2 = nc.dram_tensor("w2", [L, F, D], bf16, kind="ExternalInput").ap()
    d_wo = nc.dram_tensor("wout", [D, VC], bf16, kind="ExternalInput").ap()
    d_pp = nc.dram_tensor("pp", [L, 128, 72], f32, kind="ExternalInput").ap()
    d_mask = nc.dram_tensor("maskt", [BC, S, S], bf16, kind="ExternalInput").ap()
    d_ones = nc.dram_tensor("ones", [128, 128], f32r, kind="ExternalInput").ap()
    d_ident = nc.dram_tensor("ident", [128, 128], f32r, kind="ExternalInput").ap()
    d_bvbc = (nc.dram_tensor("bvbc", [L, 128, D], f32, kind="ExternalInput").ap()
              if bv_nz else None)
    d_bout = (nc.dram_tensor("boutbc", [128, VC], f32, kind="ExternalInput").ap()
              if bout_nz else None)
    d_out = nc.dram_tensor("out", [NCORE * NT, VC], bf16,
                           kind="ExternalOutput").ap()

    with tile.TileContext(nc) as tc, ExitStack() as top, \
            nc.allow_low_precision(reason="fp32r matmul tiles"):
        # psum pools shared across phases via tags (4+4 banks)
        pps = top.enter_context(tc.tile_pool(name="pps", bufs=5, space="PSUM"))
        ppa = top.enter_context(tc.tile_pool(name="ppa", bufs=3, space="PSUM"))
        pdram = top.enter_context(tc.tile_pool(name="pdram", bufs=1, space="DRAM"))
        pconst = top.enter_context(tc.tile_pool(name="pconst", bufs=1))

        ones_sb = pconst.tile([128, 128], f32r, tag="ones")
        nc.sync.dma_start(ones_sb[:], d_ones[:])
        ident_sb = pconst.tile([128, 128], f32r, tag="ident")
        nc.sync.dma_start(ident_sb[:], d_ident[:])
        eps_sb = pconst.tile([128, 1], f32, tag="eps")
        nc.vector.memset(eps_sb[:], EPS)
        nones_sb = pconst.tile([1, 128], f32r, tag="nones")
        nc.vector.memset(nones_sb[:, :].bitcast(mybir.dt.uint32), 0xBF800000)
        onesb_sb = pconst.tile([128, 1], bf16, tag="onesb")
        nc.vector.memset(onesb_sb[:], 1.0)
        identb_sb = pconst.tile([128, 128], bf16, tag="identb")
        nc.vector.tensor_copy(identb_sb[:], ident_sb[:])

        def mmr(out, lhsT, rhs, **kw):
            nc.tensor.matmul(out, lhsT, rhs, **kw)

        def _emit(rep):
            # dram staging for collectives
            ag_in = pdram.tile([D, NT], bf16, name=f"agin{rep}")
            ag_out = pdram.tile([NCORE, D, NT], bf16,
                                addr_space="Local" if sim_mode else "Shared",
                                name=f"agout{rep}")
            z_in = [pdram.tile([128, ZGRP], f32, name=f"zin{rep}_{g}")
                    for g in range(N_ZG)]
            z_out = [pdram.tile([128, ZGRP], f32, name=f"zout{rep}_{g}",
                                addr_space="Local" if sim_mode else "Shared")
                     for g in range(N_ZG)]

            with ExitStack() as lay:
                px = lay.enter_context(tc.tile_pool(name=f"px{rep}", bufs=20))
                pqk = lay.enter_context(tc.tile_pool(name=f"pqk{rep}", bufs=13))
                pv = lay.enter_context(tc.tile_pool(name=f"pv{rep}", bufs=5))
                patt = lay.enter_context(tc.tile_pool(name=f"patt{rep}", bufs=8))
                pescr = lay.enter_context(tc.tile_pool(name=f"pescr{rep}", bufs=4))
                pocat = lay.enter_context(tc.tile_pool(name=f"pocat{rep}", bufs=5))
                ph = lay.enter_context(tc.tile_pool(name=f"ph{rep}", bufs=26))
                plns = lay.enter_context(tc.tile_pool(name=f"plns{rep}", bufs=6))
                pmask = lay.enter_context(tc.tile_pool(name=f"pmask{rep}", bufs=4))
                pwmed = lay.enter_context(tc.tile_pool(name=f"pwmed{rep}", bufs=12))
                pwsm = lay.enter_context(tc.tile_pool(name=f"pwsm{rep}", bufs=18))
                pw2 = lay.enter_context(tc.tile_pool(name=f"pw2{rep}", bufs=26))
                ppp = lay.enter_context(tc.tile_pool(name=f"ppp{rep}", bufs=2))
                pst = lay.enter_context(tc.tile_pool(name=f"pst{rep}", bufs=12))
                pbv = (lay.enter_context(tc.tile_pool(name=f"pbv{rep}", bufs=2))
                       if bv_nz else None)

                # initial x (transposed [D, NT] bf16)
                xt = []
                for k in range(KD):
                    t = px.tile([128, NT], bf16, tag="x")
                    nc.sync.dma_start(t[:], d_x0[k * 128:(k + 1) * 128, :])
                    xt.append(t)

                # mask tiles (resident; maskT[b][t,s] layout)
                mk = {}
                for b in range(BC):
                    for ti, (t0, tl) in enumerate(LT):
                        m = pmask.tile([128, S], bf16, tag="mask")
                        nc.sync.dma_start(m[:tl, :], d_mask[b, t0:t0 + tl, :])
                        mk[(b, ti)] = m

                def ln_stats(src):
                    """col-sums of x and x^2 -> (mu, var+eps) SBUF rows."""
                    ps_mu = pps.tile([1, NT], f32, tag="mm")
                    for k in range(KD):
                        mmr(ps_mu[:], onesb_sb[:, 0:1], src[k][:],
                            start=(k == 0), stop=(k == KD - 1))
                    mu = pst.tile([1, NT], f32, tag="row")
                    nc.vector.tensor_scalar_mul(mu[:], ps_mu[:], 1.0 / D)
                    ps_sq = pps.tile([1, NT], f32, tag="mm")
                    for k in range(KD):
                        s = plns.tile([128, NT], bf16, tag="lnscr")
                        nc.gpsimd.tensor_mul(s[:], src[k][:], src[k][:])
                        mmr(ps_sq[:], onesb_sb[:, 0:1], s[:],
                            start=(k == 0), stop=(k == KD - 1))
                    var = pst.tile([1, NT], f32, tag="row")
                    nc.vector.tensor_scalar(var[:], ps_sq[:], 1.0 / D,
                                            EPS, OP.mult, OP.add)
                    return mu, var

                def ln_rows(stats):
                    """rsqrt/mean row chain on DVE+Pool -> (rinv, cc);
                    float-only Newton (seed 1/((1+v)/2)), no act tables."""
                    mu, var = stats
                    musq = pst.tile([1, NT], f32, tag="row")
                    nc.gpsimd.tensor_mul(musq[:], mu[:], mu[:])
                    nc.vector.tensor_sub(var[:], var[:], musq[:])
                    sh2 = pst.tile([1, NT], f32, tag="row")
                    nc.vector.tensor_scalar(sh2[:], var[:], 0.5, 0.5,
                                            OP.mult, OP.add)
                    seed = pst.tile([1, NT], f32, tag="row")
                    nc.vector.reciprocal(seed[:], sh2[:])
                    hv = pst.tile([1, NT], f32, tag="row")
                    nc.gpsimd.tensor_scalar_mul(hv[:], var[:], -0.5)
                    t2 = pst.tile([1, NT], f32, tag="row")
                    s2 = pst.tile([1, NT], f32, tag="row")
                    nc.vector.tensor_mul(t2[:], seed[:], seed[:])
                    nc.vector.tensor_mul(t2[:], t2[:], hv[:])
                    nc.vector.tensor_scalar_add(t2[:], t2[:], 1.5)
                    nc.vector.tensor_mul(s2[:], seed[:], t2[:])
                    rinv = pst.tile([1, NT], f32r, tag="row")
                    nc.vector.tensor_mul(t2[:], s2[:], s2[:])
                    nc.vector.tensor_mul(t2[:], t2[:], hv[:])
                    nc.vector.tensor_scalar_add(t2[:], t2[:], 1.5)
                    nc.vector.tensor_mul(rinv[:], s2[:], t2[:])
                    cc = pst.tile([1, NT], f32r, tag="row")
                    nc.gpsimd.tensor_mul(cc[:], mu[:], rinv[:])
                    return rinv, cc

                def ln_apply(rows, src, s_col, b_col, pp_sb):
                    rinv, cc = rows
                    pa = pps.tile([128, NT], f32, tag="mm")
                    mmr(pa[:], ones_sb[0:1, :], rinv[:], start=True, stop=True)
                    pc = pps.tile([128, NT], f32, tag="mm")
                    mmr(pc[:], nones_sb[:, :], cc[:], start=True, stop=True)
                    out = []
                    for k in range(KD):
                        y = px.tile([128, NT], bf16, tag="x")
                        if ln_ident:
                            nc.vector.tensor_mul(y[:], src[k][:], pa[:])
                            nc.vector.tensor_add(y[:], y[:], pc[:])
                        else:
                            t1 = plns.tile([128, NT], f32, tag="lnscr2")
                            nc.vector.tensor_mul(t1[:], src[k][:], pa[:])
                            nc.vector.tensor_add(t1[:], t1[:], pc[:])
                            nc.vector.tensor_scalar(
                                y[:], t1[:],
                                pp_sb[:, s_col + k:s_col + k + 1],
                                pp_sb[:, b_col + k:b_col + k + 1],
                                OP.mult, OP.add)
                        out.append(y)
                    return out

                for l in range(n_layers):
                    pp_sb = ppp.tile([128, 72], f32, tag="pp")
                    nc.sync.dma_start(pp_sb[:], d_pp[l])

                    wq_sb, wk_sb, wv_sb = [], [], []
                    for k in range(KD):
                        t = pwsm.tile([128, D], bf16, tag="wsm")
                        nc.sync.dma_start(t[:], d_wqk[l, k * 128:(k + 1) * 128,
                                                      0:D])
                        wq_sb.append(t)
                    for k in range(KD):
                        t = pwsm.tile([128, D], bf16, tag="wsm")
                        nc.sync.dma_start(t[:], d_wqk[l, k * 128:(k + 1) * 128,
                                                      D:2 * D])
                        wk_sb.append(t)
                    for k in range(KD):
                        t = pwsm.tile([128, D], bf16, tag="wsm")
                        nc.sync.dma_start(t[:], d_wv[l, k * 128:(k + 1) * 128, :])
                        wv_sb.append(t)
                    if bv_nz:
                        bv_sb = pbv.tile([128, D], f32, tag="bv")
                        nc.sync.dma_start(bv_sb[:], d_bvbc[l])

                    # ---- Q,K projections ([head*hd, tok] layout) ----
                    qk_sb = []
                    for m in range(12):
                        w = wq_sb if m < 6 else wk_sb
                        mm = m % 6
                        ps = pps.tile([128, NT], f32, tag="mm")
                        for k in range(KD):
                            mmr(ps[:], w[k][:, mm * 128:(mm + 1) * 128],
                                xt[k][:], start=(k == 0), stop=(k == KD - 1))
                        sb = pqk.tile([128, NT], bf16, tag="qk")
                        nc.scalar.activation(sb[:], ps[:], AF.Identity,
                                             bias=pp_sb[:, 30 + m:31 + m])
                        qk_sb.append(sb)

                    # ---- V projection (natural layout, per-batch tiles) ----
                    v_sb = {}
                    for b in range(BC):
                        for ti, (t0, tl) in enumerate(LT):
                            g0 = b * S + t0
                            vt = pv.tile([128, 12 * 65], bf16, tag="v")
                            vt3 = vt[:, :].rearrange("p (h e) -> p h e", e=65)
                            nc.vector.memset(vt3[:tl, :, 64:65], 1.0)
                            for n in range(2):
                                ps = ppa.tile([128, 384], f32, tag="aux")
                                for k in range(KD):
                                    mmr(ps[:tl, :], xt[k][:, g0:g0 + tl],
                                        wv_sb[k][:, n * 384:(n + 1) * 384],
                                        start=(k == 0), stop=(k == KD - 1))
                                if bv_nz:
                                    nc.vector.tensor_add(
                                        ps[:tl, :], ps[:tl, :],
                                        bv_sb[:tl, n * 384:(n + 1) * 384])
                                ps3 = ps[:, :].rearrange("p (h e) -> p h e",
                                                         e=64)
                                nc.scalar.activation(
                                    vt3[:tl, n * 6:(n + 1) * 6, 0:64],
                                    ps3[:tl, :, :], AF.Identity)
                            v_sb[(b, ti)] = vt

                    # ---- attention ----
                    ocat = {}
                    for b in range(BC):
                        for st in range(2):
                            ocat[(b, st)] = pocat.tile([128, D], bf16,
                                                       tag="ocat", name="ocat")

                    for b in range(BC):
                        for half in range(2):
                            atts = {}
                            for hh in range(6):
                                h = half * 6 + hh
                                hq = qk_sb[h // 2]
                                hk = qk_sb[6 + h // 2]
                                hb = (h % 2) * 64
                                att = {}
                                for ti, (t0, tl) in enumerate(LT):
                                    ps = pps.tile([128, NT], f32, tag="mm")
                                    nc.tensor.matmul(
                                        ps[:tl, t0:S],
                                        hk[hb:hb + 64,
                                           b * S + t0:b * S + t0 + tl],
                                        hq[hb:hb + 64, b * S + t0:b * S + S],
                                        start=True, stop=True)
                                    esc = pescr.tile([128, S], f32, tag="escr")
                                    nc.scalar.activation(
                                        esc[:tl, t0:S], ps[:tl, t0:S],
                                        AF.Exp, scale=ATT_SCALE)
                                    at = patt.tile([128, S], bf16, tag="att")
                                    nc.gpsimd.tensor_mul(
                                        at[:tl, t0:S], esc[:tl, t0:S],
                                        mk[(b, ti)][:tl, t0:S])
                                    att[ti] = at
                                atts[hh] = att
                            for st in range(2):
                                s0, sl = LT[st]
                                tis = [ti for ti, (t0, tl) in enumerate(LT)
                                       if t0 < s0 + sl]
                                po6 = ppa.tile([128, 390], f32, tag="aux",
                                               name="po6")
                                for hh in range(6):
                                    h = half * 6 + hh
                                    for i, ti in enumerate(tis):
                                        t0, tl = LT[ti]
                                        nc.tensor.matmul(
                                            po6[:sl, hh * 65:(hh + 1) * 65],
                                            atts[hh][ti][:tl, s0:s0 + sl],
                                            v_sb[(b, ti)][:tl,
                                                          h * 65:(h + 1) * 65],
                                            start=(i == 0),
                                            stop=(i == len(tis) - 1))
                                po63 = po6[:, :].rearrange("p (h e) -> p h e",
                                                           e=65)
                                rz6 = pst.tile([128, 6], f32, tag="row")
                                nc.vector.reciprocal(rz6[:sl, :],
                                                     po63[:sl, :, 64:65])
                                for hh in range(6):
                                    h = half * 6 + hh
                                    if hh % 2 == 0:
                                        nc.vector.tensor_scalar_mul(
                                            ocat[(b, st)][:sl,
                                                          h * 64:(h + 1) * 64],
                                            po63[:sl, hh, 0:64],
                                            rz6[:sl, hh:hh + 1])
                                    else:
                                        nc.scalar.activation(
                                            ocat[(b, st)][:sl,
                                                          h * 64:(h + 1) * 64],
                                            po63[:sl, hh, 0:64], AF.Identity,
                                            scale=rz6[:sl, hh:hh + 1])

                    # ---- transpose o + residual add -> x2t ----
                    x2t = [px.tile([128, NT], bf16, tag="x", name="x2t")
                           for _ in range(KD)]
                    for b in range(BC):
                        for st in range(2):
                            s0, sl = LT[st]
                            g0 = b * S + s0
                            for k in range(KD):
                                pt = ppa.tile([128, 128], bf16, tag="aux")
                                nc.tensor.transpose(
                                    pt[:, :sl],
                                    ocat[(b, st)][:sl, k * 128:(k + 1) * 128],
                                    identb_sb[:sl, :sl])
                                nc.vector.tensor_add(x2t[k][:, g0:g0 + sl],
                                                     xt[k][:, g0:g0 + sl],
                                                     pt[:, :sl])

                    st1 = ln_stats(x2t)
                    r1 = ln_rows(st1)
                    y1t = ln_apply(r1, x2t, 0, 6, pp_sb)

                    # ---- FFN1 (+bias+relu, bf16 out) ----
                    ht = []
                    for half in range(2):
                        w1h = []
                        for k in range(KD):
                            t = pwmed.tile([128, F // 2], bf16, tag="wmed")
                            nc.sync.dma_start(
                                t[:], d_w1[l, k * 128:(k + 1) * 128,
                                           half * (F // 2):
                                           (half + 1) * (F // 2)])
                            w1h.append(t)
                        for m in range(12):
                            fm = half * 12 + m
                            ps = pps.tile([128, NT], f32, tag="mm")
                            for k in range(KD):
                                mmr(ps[:], w1h[k][:, m * 128:(m + 1) * 128],
                                    y1t[k][:], start=(k == 0),
                                    stop=(k == KD - 1))
                            hb16 = ph.tile([128, NT], bf16, tag="h")
                            nc.scalar.activation(hb16[:], ps[:], AF.Relu,
                                                 bias=pp_sb[:, 42 + fm:43 + fm])
                            ht.append(hb16)

                    # ---- FFN2 (bf16) + residual -> x3t ----
                    x3t = []
                    for half in range(2):
                        w2h = []
                        for k in range(KF):
                            t = pw2.tile([128, 384], bf16, tag="w2")
                            nc.sync.dma_start(
                                t[:], d_w2[l, k * 128:(k + 1) * 128,
                                           half * 384:(half + 1) * 384])
                            w2h.append(t)
                        for m in range(3):
                            dm = half * 3 + m
                            ps = pps.tile([128, NT], f32, tag="mm")
                            for k in range(KF):
                                nc.tensor.matmul(
                                    ps[:], w2h[k][:, m * 128:(m + 1) * 128],
                                    ht[k][:], start=(k == 0),
                                    stop=(k == KF - 1))
                            x3 = px.tile([128, NT], bf16, tag="x")
                            nc.vector.tensor_add(x3[:], ps[:], y1t[dm][:])
                            if b2_nz:
                                nc.vector.tensor_scalar_add(
                                    x3[:], x3[:], pp_sb[:, 24 + dm:25 + dm])
                            x3t.append(x3)

                    st2 = ln_stats(x3t)
                    r2 = ln_rows(st2)
                    xt = ln_apply(r2, x3t, 12, 18, pp_sb)

                # ship final x (bf16) straight to DRAM for the AllGather
                for k in range(KD):
                    nc.sync.dma_start(ag_in[k * 128:(k + 1) * 128, :], xt[k][:])

            # ================= final: AllGather + vocab-sharded projection ======
            if not do_final:
                t_dump = pconst.tile([128, 8], bf16, tag="dump", name=f"dump{rep}")
                nc.sync.dma_start(t_dump[:], ag_in[0:128, 0:8])
                nc.sync.dma_start(d_out[0:128, 0:8], t_dump[:])
                return
            with ExitStack() as fin:
                if sim_mode:
                    for c in range(NCORE):
                        nc.sync.dma_start(ag_out[c], ag_in[:])
                else:
                    nc.gpsimd.collective_compute(
                        "AllGather", mybir.AluOpType.bypass,
                        replica_groups=[list(range(NCORE))],
                        ins=[ag_in.opt()], outs=[ag_out.opt()])

                pwo = fin.enter_context(tc.tile_pool(name=f"pwo{rep}", bufs=6))
                pxa = fin.enter_context(tc.tile_pool(name=f"pxa{rep}", bufs=12))
                pstrip = fin.enter_context(tc.tile_pool(name=f"pstrip{rep}", bufs=8))
                pstg = fin.enter_context(tc.tile_pool(name=f"pstg{rep}", bufs=8))
                pz = fin.enter_context(tc.tile_pool(name=f"pz{rep}", bufs=N_MT + 2))
                pzr = fin.enter_context(tc.tile_pool(name=f"pzr{rep}", bufs=6))
                pbo = (fin.enter_context(tc.tile_pool(name=f"pbo{rep}", bufs=1))
                       if bout_nz else None)

                wo_sb = []
                for k in range(KD):
                    t = pwo.tile([128, VC], bf16, tag="wo")
                    nc.sync.dma_start(t[:], d_wo[k * 128:(k + 1) * 128, :])
                    wo_sb.append(t)
                if bout_nz:
                    bo_sb = pbo.tile([128, VC], f32, tag="bo")
                    nc.sync.dma_start(bo_sb[:], d_bout[:])

                zg_sb = pzr.tile([128, N_MT], f32, tag="zg")
                nc.vector.memset(zg_sb[:], 1.0)
                rz_sb = pzr.tile([128, N_MT], f32, tag="rzf")

                strips = {}
                zp = {}
                xa_c = {}
                for mt in range(N_MT):
                    c, j = mt // 4, mt % 4
                    m0, ml = MT_LOC[j]
                    if c not in xa_c:
                        ts = []
                        for k in range(KD):
                            t = pxa.tile([128, NT], bf16, tag="xa")
                            nc.sync.dma_start(t[:],
                                              ag_out[c, k * 128:(k + 1) * 128, :])
                            ts.append(t)
                        xa_c[c] = ts
                    xa = xa_c[c]
                    strip = pstrip.tile([128, VC], bf16, tag="strip")
                    strips[mt] = strip
                    zpt = pz.tile([128, len(VCH)], f32, tag="zp")
                    zp[mt] = zpt
                    for vi, (v0, vw) in enumerate(VCH):
                        ps = ppa.tile([128, 512], f32, tag="aux")
                        for k in range(KD):
                            nc.tensor.matmul(ps[:ml, :vw],
                                             xa[k][:, m0:m0 + ml],
                                             wo_sb[k][:, v0:v0 + vw],
                                             start=(k == 0), stop=(k == KD - 1))
                        if bout_nz:
                            nc.vector.tensor_add(ps[:ml, :vw], ps[:ml, :vw],
                                                 bo_sb[:ml, v0:v0 + vw])
                        nc.scalar.activation(strip[:ml, v0:v0 + vw], ps[:ml, :vw],
                                             AF.Exp,
                                             accum_out=zpt[:ml, vi:vi + 1])
                    nc.vector.tensor_reduce(zg_sb[:ml, mt:mt + 1], zpt[:ml, :],
                                            AX.X, OP.add)

                    if mt % ZGRP == ZGRP - 1:
                        g = mt // ZGRP
                        nc.sync.dma_start(z_in[g][:],
                                          zg_sb[:, g * ZGRP:(g + 1) * ZGRP])
                        if sim_mode:
                            nc.sync.dma_start(z_out[g][:], z_in[g][:])
                        else:
                            nc.gpsimd.collective_compute(
                                "AllReduce", mybir.AluOpType.add,
                                replica_groups=[list(range(NCORE))],
                                ins=[z_in[g].opt()], outs=[z_out[g].opt()])
                        zr = pzr.tile([128, ZGRP], f32, tag="zred")
                        nc.sync.dma_start(zr[:], z_out[g][:])
                        nc.vector.tensor_scalar_add(zr[:], zr[:], -float(N_VPAD))
                        nc.vector.reciprocal(
                            rz_sb[:, g * ZGRP:(g + 1) * ZGRP], zr[:])
                        for mt2 in range(g * ZGRP, (g + 1) * ZGRP):
                            c2, j2 = mt2 // 4, mt2 % 4
                            m02, ml2 = MT_LOC[j2]
                            r0 = c2 * NT + m02
                            for ci, (v0, vw) in enumerate(VCH):
                                so = pstg.tile([128, 512], bf16, tag="stg")
                                eng = nc.vector if ci % 2 == 0 else nc.gpsimd
                                eng.tensor_scalar_mul(
                                    so[:ml2, :vw],
                                    strips[mt2][:ml2, v0:v0 + vw],
                                    rz_sb[:ml2, mt2:mt2 + 1])
                                nc.sync.dma_start(d_out[r0:r0 + ml2, v0:v0 + vw],
                                                  so[:ml2, :vw])
                            del strips[mt2]

        for rep in range(repeat):
            _emit(rep)

    nc.compile()
    return nc


def _get_nc(flags, repeat=1, sim_mode=False, n_layers=L, do_final=True,
            ablate=()):
    key = (flags, repeat, sim_mode, n_layers, do_final, tuple(ablate))
    if key not in _CACHE:
        _CACHE[key] = _build(flags, repeat, sim_mode, n_layers, do_final,
                             ablate)
    return _CACHE[key]


def _prep(inputs):
    """Host-side preprocessing -> (per-core in_maps, specialization flags)."""
    x_img = np.asarray(inputs["image_token"], np.float32)
    tok = np.asarray(inputs["text_token"])
    tmask = np.asarray(inputs["text_mask"])
    temb = np.asarray(inputs["text_emb"], np.float32)
    semb = np.asarray(inputs["sep_emb"], np.float32)
    Wq = np.asarray(inputs["Wq"], np.float32)
    bq = np.asarray(inputs["bq"], np.float32)
    Wk = np.asarray(inputs["Wk"], np.float32)
    bk = np.asarray(inputs["bk"], np.float32)
    Wv = np.asarray(inputs["Wv"], np.float32)
    bv = np.asarray(inputs["bv"], np.float32)
    ln1_s = np.asarray(inputs["ln1_s"], np.float32)
    ln1_b = np.asarray(inputs["ln1_b"], np.float32)
    W1 = np.asarray(inputs["W1"], np.float32)
    b1 = np.asarray(inputs["b1"], np.float32)
    W2 = np.asarray(inputs["W2"], np.float32)
    b2 = np.asarray(inputs["b2"], np.float32)
    ln2_s = np.asarray(inputs["ln2_s"], np.float32)
    ln2_b = np.asarray(inputs["ln2_b"], np.float32)
    Wout = np.asarray(inputs["Wout"], np.float32)
    bout = np.asarray(inputs["bout"], np.float32)

    # x0 = [img | sep | emb[tokens]]
    x0 = np.concatenate(
        [x_img, np.broadcast_to(semb[None], (B, 1, D)), temb[tok]], axis=1)

    # maskT[b][t,s] = (t<=s) & combined[b,t]
    comb = np.concatenate(
        [np.ones((B, S - TXT), bool), tmask.astype(bool)], axis=1)
    tril_t = np.tril(np.ones((S, S), bool)).T  # [t,s]: t<=s
    maskt = (tril_t[None] & comb[:, :, None]).astype(ml_dtypes.bfloat16)

    # packed weights (bf16)
    wqk = np.ascontiguousarray(np.concatenate([
        Wq.transpose(0, 2, 1, 3).reshape(L, D, D),
        Wk.transpose(0, 2, 1, 3).reshape(L, D, D)],
        axis=2).astype(ml_dtypes.bfloat16))
    wv = np.ascontiguousarray(
        Wv.transpose(0, 2, 1, 3).reshape(L, D, D).astype(ml_dtypes.bfloat16))
    w1 = W1.astype(ml_dtypes.bfloat16)
    w2 = W2.astype(ml_dtypes.bfloat16)

    # per-partition params: [L,128,72]
    pp = np.zeros((L, 128, 72), np.float32)

    def put(dst0, arr):  # arr [L, n*128]
        n = arr.shape[1] // 128
        pp[:, :, dst0:dst0 + n] = arr.reshape(L, n, 128).transpose(0, 2, 1)

    put(0, ln1_s); put(6, ln1_b); put(12, ln2_s); put(18, ln2_b); put(24, b2)
    put(30, np.concatenate([bq.reshape(L, D), bk.reshape(L, D)], axis=1))
    put(42, b1)

    wo_pad = np.zeros((D, V_PAD), ml_dtypes.bfloat16)
    wo_pad[:, :V] = Wout.astype(ml_dtypes.bfloat16)
    bout_pad = np.zeros(V_PAD, np.float32)
    bout_pad[:V] = bout

    ln_ident = bool(np.all(ln1_s == 1.0) and np.all(ln1_b == 0.0)
                    and np.all(ln2_s == 1.0) and np.all(ln2_b == 0.0))
    flags = (bool(np.any(bv)), bool(np.any(b2)), bool(np.any(bout)), ln_ident)

    ones = np.ones((128, 128), np.float32)
    ident = np.eye(128, dtype=np.float32)

    in_maps = []
    for c in range(NCORE):
        m = {
            "x0t": np.ascontiguousarray(
                x0[c * BC:(c + 1) * BC].reshape(NT, D).T
                .astype(ml_dtypes.bfloat16)),
            "wqk": wqk, "wv": wv, "w1": w1, "w2": w2,
            "wout": np.ascontiguousarray(wo_pad[:, c * VC:(c + 1) * VC]),
            "pp": pp,
            "maskt": np.ascontiguousarray(maskt[c * BC:(c + 1) * BC]),
            "ones": ones, "ident": ident,
        }
        if flags[0]:
            m["bvbc"] = np.ascontiguousarray(np.broadcast_to(
                bv.reshape(L, 1, D), (L, 128, D)))
        if flags[2]:
            m["boutbc"] = np.ascontiguousarray(np.broadcast_to(
                bout_pad[c * VC:(c + 1) * VC][None], (128, VC)))
        in_maps.append(m)
    return in_maps, flags


def kernel(**inputs):
    from concourse.bass_utils import run_bass_kernel_spmd
    in_maps, flags = _prep(inputs)
    nc = _get_nc(flags)
    res = run_bass_kernel_spmd(nc, in_maps, list(range(NCORE)))
    full = np.concatenate([res.results[c]["out"] for c in range(NCORE)], axis=1)
    return np.ascontiguousarray(
        full[:, :V].astype(np.float32).reshape(B, S, V))



# revision 40
# speedup vs baseline: 2.0097x; 1.8895x over previous
"""Trainium2 Bass kernel for a 6-layer caption-generator transformer.

Sharding: data-parallel over batch (16 -> 2 per core) for the 6 transformer
layers; vocab-sharded final projection + softmax (50257 -> 6283 cols/core)
with an AllGather of final hidden states and pipelined grouped AllReduces
of the softmax denominators.

Self-contained: hardcodes all shapes; builds + compiles the Bass/Tile
program on first call (cached) and runs it on 8 NeuronCores via
run_bass_kernel_spmd.
"""

import numpy as np
import ml_dtypes
from contextlib import ExitStack

# ---- model dims (hardcoded from the problem spec) ----
B, IMG, TXT = 16, 197, 24
S = IMG + TXT + 1          # 222
D, H, L, V = 768, 12, 6, 50257
HD = D // H                # 64
F = 4 * D                  # 3072
EPS = 1e-5
NCORE = 8
BC = B // NCORE            # 2 batches per core
NT = BC * S                # 444 tokens per core
KD = D // 128              # 6 k-tiles over D
KF = F // 128              # 24 k-tiles over F
ATT_SCALE = 1.0 / float(np.sqrt(np.float32(HD)))

# local (per-batch) token tiles: (offset, len)
LT = [(0, 128), (128, S - 128)]          # [(0,128),(128,94)]
# per-core token tiles within the 444-token block
MT_LOC = [(0, 128), (128, 128), (256, 128), (384, NT - 384)]

# vocab sharding
VC = 6283                   # vocab cols per core (8*6283 = 50264 >= 50257)
V_PAD = VC * NCORE          # 50264
N_VPAD = V_PAD - V          # 7 padded cols (wout=0 -> logits 0 -> exp 1)
VCH = [(i * 512, 512) for i in range(12)] + [(6144, VC - 6144)]   # 12x512+139

N_MT = NCORE * len(MT_LOC)  # 32 token m-tiles over the gathered 3552 tokens
ZGRP = 4                    # m-tiles per Z AllReduce group
N_ZG = N_MT // ZGRP         # 8 groups

_CACHE = {}


def _build(flags, repeat=1, sim_mode=False, n_layers=L, do_final=True,
           ablate=()):
    import concourse.bass as bass
    import concourse.tile as tile
    import concourse.mybir as mybir
    from concourse import bacc

    f32 = mybir.dt.float32
    f32r = mybir.dt.float32r
    bf16 = mybir.dt.bfloat16
    AF = mybir.ActivationFunctionType
    OP = mybir.AluOpType
    AX = mybir.AxisListType

    bv_nz, b2_nz, bout_nz, ln_ident = flags

    nc = bacc.Bacc("TRN2", target_bir_lowering=False, debug=False,
                   num_devices=NCORE)

    # ---- DRAM I/O ----
    d_x0 = nc.dram_tensor("x0t", [D, NT], f32r, kind="ExternalInput").ap()
    d_wqk = nc.dram_tensor("wqk", [L, D, 2 * D], f32r, kind="ExternalInput").ap()
    d_wv = nc.dram_tensor("wv", [L, D, D], f32r, kind="ExternalInput").ap()
    d_w1 = nc.dram_tensor("w1", [L, D, F], f32r, kind="ExternalInput").ap()
    d_w2 = nc.dram_tensor("w2", [L, F, D], bf16, kind="ExternalInput").ap()
    d_wo = nc.dram_tensor("wout", [D, VC], bf16, kind="ExternalInput").ap()
    d_pp = nc.dram_tensor("pp", [L, 128, 72], f32, kind="ExternalInput").ap()
    d_mask = nc.dram_tensor("maskt", [BC, S, S], bf16, kind="ExternalInput").ap()
    d_ones = nc.dram_tensor("ones", [128, 128], f32r, kind="ExternalInput").ap()
    d_ident = nc.dram_tensor("ident", [128, 128], f32r, kind="ExternalInput").ap()
    d_bvbc = (nc.dram_tensor("bvbc", [L, 128, D], f32, kind="ExternalInput").ap()
              if bv_nz else None)
    d_bout = (nc.dram_tensor("boutbc", [128, VC], f32, kind="ExternalInput").ap()
              if bout_nz else None)
    d_out = nc.dram_tensor("out", [NCORE * NT, VC], bf16,
                           kind="ExternalOutput").ap()

    with tile.TileContext(nc) as tc, ExitStack() as top, \
            nc.allow_low_precision(reason="fp32r matmul tiles"):
        # psum pools shared across phases via tags (4+4 banks)
        pps = top.enter_context(tc.tile_pool(name="pps", bufs=5, space="PSUM"))
        ppa = top.enter_context(tc.tile_pool(name="ppa", bufs=3, space="PSUM"))
        pdram = top.enter_context(tc.tile_pool(name="pdram", bufs=1, space="DRAM"))
        pconst = top.enter_context(tc.tile_pool(name="pconst", bufs=1))

        ones_sb = pconst.tile([128, 128], f32r, tag="ones")
        nc.sync.dma_start(ones_sb[:], d_ones[:])
        ident_sb = pconst.tile([128, 128], f32r, tag="ident")
        nc.sync.dma_start(ident_sb[:], d_ident[:])
        eps_sb = pconst.tile([128, 1], f32, tag="eps")
        nc.vector.memset(eps_sb[:], EPS)

        def mmr(out, lhsT, rhs, **kw):
            nc.tensor.matmul(out, lhsT, rhs, **kw)

        def _emit(rep):
            # dram staging for collectives
            ag_in = pdram.tile([D, NT], bf16, name=f"agin{rep}")
            ag_out = pdram.tile([NCORE, D, NT], bf16,
                                addr_space="Local" if sim_mode else "Shared",
                                name=f"agout{rep}")
            z_in = [pdram.tile([128, ZGRP], f32, name=f"zin{rep}_{g}")
                    for g in range(N_ZG)]
            z_out = [pdram.tile([128, ZGRP], f32, name=f"zout{rep}_{g}",
                                addr_space="Local" if sim_mode else "Shared")
                     for g in range(N_ZG)]

            with ExitStack() as lay:
                px = lay.enter_context(tc.tile_pool(name=f"px{rep}", bufs=15))
                pqk = lay.enter_context(tc.tile_pool(name=f"pqk{rep}", bufs=13))
                pv = lay.enter_context(tc.tile_pool(name=f"pv{rep}", bufs=5))
                patt = lay.enter_context(tc.tile_pool(name=f"patt{rep}", bufs=8))
                pescr = lay.enter_context(tc.tile_pool(name=f"pescr{rep}", bufs=4))
                pocat = lay.enter_context(tc.tile_pool(name=f"pocat{rep}", bufs=5))
                ph = lay.enter_context(tc.tile_pool(name=f"ph{rep}", bufs=26))
                plns = lay.enter_context(tc.tile_pool(name=f"plns{rep}", bufs=5))
                pmask = lay.enter_context(tc.tile_pool(name=f"pmask{rep}", bufs=4))
                pwmed = lay.enter_context(tc.tile_pool(name=f"pwmed{rep}", bufs=6))
                pwsm = lay.enter_context(tc.tile_pool(name=f"pwsm{rep}", bufs=10))
                pw2 = lay.enter_context(tc.tile_pool(name=f"pw2{rep}", bufs=26))
                ppp = lay.enter_context(tc.tile_pool(name=f"ppp{rep}", bufs=2))
                pst = lay.enter_context(tc.tile_pool(name=f"pst{rep}", bufs=8))
                pbv = (lay.enter_context(tc.tile_pool(name=f"pbv{rep}", bufs=2))
                       if bv_nz else None)

                # initial x (transposed [D, NT])
                xt = []
                for k in range(KD):
                    t = px.tile([128, NT], f32r, tag="x")
                    nc.sync.dma_start(t[:], d_x0[k * 128:(k + 1) * 128, :])
                    xt.append(t)

                # mask tiles (resident; maskT[b][t,s] layout)
                mk = {}
                for b in range(BC):
                    for ti, (t0, tl) in enumerate(LT):
                        m = pmask.tile([128, S], bf16, tag="mask")
                        nc.sync.dma_start(m[:tl, :], d_mask[b, t0:t0 + tl, :])
                        mk[(b, ti)] = m

                for l in range(n_layers):
                    pp_sb = ppp.tile([128, 72], f32, tag="pp")
                    nc.sync.dma_start(pp_sb[:], d_pp[l])

                    x2t = xt if "attn" in ablate else None
                    # ---- Q,K projections ([head*hd, tok] layout) ----
                    wq_sb, wk_sb = [], []
                    for k in range(KD if x2t is None else 0):
                        t = pwsm.tile([128, D], f32r, tag="wsm")
                        nc.sync.dma_start(t[:], d_wqk[l, k * 128:(k + 1) * 128, 0:D])
                        wq_sb.append(t)
                    for k in range(KD):
                        t = pwsm.tile([128, D], f32r, tag="wsm")
                        nc.sync.dma_start(t[:], d_wqk[l, k * 128:(k + 1) * 128, D:2 * D])
                        wk_sb.append(t)

                    qk_sb = []
                    for m in range(12 if x2t is None else 0):
                        w = wq_sb if m < 6 else wk_sb
                        mm = m % 6
                        ps = pps.tile([128, NT], f32, tag="mm")
                        for k in range(KD):
                            mmr(ps[:], w[k][:, mm * 128:(mm + 1) * 128], xt[k][:],
                                start=(k == 0), stop=(k == KD - 1))
                        sb = pqk.tile([128, NT], bf16, tag="qk")
                        nc.scalar.activation(sb[:], ps[:], AF.Identity,
                                             bias=pp_sb[:, 30 + m:31 + m])
                        qk_sb.append(sb)

                    # ---- V projection (natural layout, per-batch token tiles) ----
                    wv_sb = []
                    for k in range(KD if x2t is None else 0):
                        t = pwsm.tile([128, D], f32r, tag="wsm")
                        nc.sync.dma_start(t[:], d_wv[l, k * 128:(k + 1) * 128, :])
                        wv_sb.append(t)
                    if bv_nz:
                        bv_sb = pbv.tile([128, D], f32, tag="bv")
                        nc.sync.dma_start(bv_sb[:], d_bvbc[l])

                    v_sb = {}
                    for b in range(BC if x2t is None else 0):
                        for ti, (t0, tl) in enumerate(LT):
                            g0 = b * S + t0
                            vt = pv.tile([128, 12 * 65], bf16, tag="v")
                            vt3 = vt[:, :].rearrange("p (h e) -> p h e", e=65)
                            nc.vector.memset(vt3[:tl, :, 64:65], 1.0)
                            for n in range(2):
                                ps = pps.tile([128, 384], f32, tag="mm")
                                for k in range(KD):
                                    mmr(ps[:tl, :], xt[k][:, g0:g0 + tl],
                                        wv_sb[k][:, n * 384:(n + 1) * 384],
                                        start=(k == 0), stop=(k == KD - 1))
                                if bv_nz:
                                    nc.vector.tensor_add(
                                        ps[:tl, :], ps[:tl, :],
                                        bv_sb[:tl, n * 384:(n + 1) * 384])
                                ps3 = ps[:, :].rearrange("p (h e) -> p h e", e=64)
                                nc.scalar.activation(
                                    vt3[:tl, n * 6:(n + 1) * 6, 0:64],
                                    ps3[:tl, :, :], AF.Identity)
                            v_sb[(b, ti)] = vt

                    # ---- attention ----
                    ocat = {}
                    for b in range(BC if x2t is None else 0):
                        for st in range(2):
                            ocat[(b, st)] = pocat.tile([128, D], f32r, tag="ocat",
                                                       name="ocat")

                    for b in range(BC if x2t is None else 0):
                        for half in range(2):
                            atts = {}
                            for hh in range(6):
                                h = half * 6 + hh
                                hq = qk_sb[h // 2]
                                hk = qk_sb[6 + h // 2]
                                hb = (h % 2) * 64
                                att = {}
                                for ti, (t0, tl) in enumerate(LT):
                                    # scoresT [t, 444 cols] (2-batch-wide rhs)
                                    ps = pps.tile([128, NT], f32, tag="mm")
                                    nc.tensor.matmul(
                                        ps[:tl, t0:S],
                                        hk[hb:hb + 64,
                                           b * S + t0:b * S + t0 + tl],
                                        hq[hb:hb + 64, b * S + t0:b * S + S],
                                        start=True, stop=True)
                                    # exp (cols s >= t0), then mask-multiply
                                    esc = pescr.tile([128, S], f32, tag="escr")
                                    nc.scalar.activation(
                                        esc[:tl, t0:S],
                                        ps[:tl, t0:S],
                                        AF.Exp, scale=ATT_SCALE)
                                    at = patt.tile([128, S], bf16, tag="att")
                                    nc.gpsimd.tensor_mul(
                                        at[:tl, t0:S], esc[:tl, t0:S],
                                        mk[(b, ti)][:tl, t0:S])
                                    att[ti] = at
                                atts[hh] = att
                            # o = attn @ [v|1]: 6 heads share one psum bank
                            for st in range(2):
                                s0, sl = LT[st]
                                tis = [ti for ti, (t0, tl) in enumerate(LT)
                                       if t0 < s0 + sl]
                                po6 = ppa.tile([128, 390], f32, tag="aux",
                                               name="po6")
                                for hh in range(6):
                                    h = half * 6 + hh
                                    for i, ti in enumerate(tis):
                                        t0, tl = LT[ti]
                                        nc.tensor.matmul(
                                            po6[:sl, hh * 65:(hh + 1) * 65],
                                            atts[hh][ti][:tl, s0:s0 + sl],
                                            v_sb[(b, ti)][:tl,
                                                          h * 65:(h + 1) * 65],
                                            start=(i == 0),
                                            stop=(i == len(tis) - 1))
                                po63 = po6[:, :].rearrange(
                                    "p (h e) -> p h e", e=65)
                                rz6 = pst.tile([128, 6], f32, tag="rz")
                                nc.vector.reciprocal(rz6[:sl, :],
                                                     po63[:sl, :, 64:65])
                                for hh in range(6):
                                    h = half * 6 + hh
                                    nc.vector.tensor_scalar_mul(
                                        ocat[(b, st)][:sl,
                                                      h * 64:(h + 1) * 64],
                                        po63[:sl, hh, 0:64],
                                        rz6[:sl, hh:hh + 1])

                    # ---- transpose o + residual add -> x2t ----
                    if x2t is None:
                        x2t = [px.tile([128, NT], f32r, tag="x", name="x2t")
                               for _ in range(KD)]
                    for b in range(BC if "attn" not in ablate else 0):
                        for st in range(2):
                            s0, sl = LT[st]
                            g0 = b * S + s0
                            for k in range(KD):
                                pt = ppa.tile([128, 128], f32r, tag="aux")
                                nc.tensor.transpose(
                                    pt[:, :sl],
                                    ocat[(b, st)][:sl, k * 128:(k + 1) * 128],
                                    ident_sb[:sl, :sl])
                                nc.vector.tensor_add(x2t[k][:, g0:g0 + sl],
                                                     xt[k][:, g0:g0 + sl],
                                                     pt[:, :sl])

                    def layernorm(src, s_col, b_col):
                        """src: KD tiles [128,NT] f32 -> normalized tiles."""
                        if "ln" in ablate:
                            return src
                        ps_mu = pps.tile([1, NT], f32, tag="mm")
                        for k in range(KD):
                            mmr(ps_mu[:], ones_sb[:, 0:1], src[k][:],
                                start=(k == 0), stop=(k == KD - 1))
                        ps_sq = pps.tile([1, NT], f32, tag="mm")
                        for k in range(KD):
                            s = plns.tile([128, NT], f32r, tag="lnscr")
                            nc.gpsimd.tensor_mul(s[:], src[k][:], src[k][:])
                            mmr(ps_sq[:], ones_sb[:, 0:1], s[:],
                                start=(k == 0), stop=(k == KD - 1))
                        mu = pst.tile([1, NT], f32, tag="row")
                        nc.vector.tensor_scalar_mul(mu[:], ps_mu[:], 1.0 / D)
                        var = pst.tile([1, NT], f32, tag="row")
                        nc.vector.tensor_scalar_mul(var[:], ps_sq[:], 1.0 / D)
                        musq = pst.tile([1, NT], f32, tag="row")
                        nc.vector.tensor_mul(musq[:], mu[:], mu[:])
                        nc.vector.tensor_sub(var[:], var[:], musq[:])
                        sd = pst.tile([1, NT], f32, tag="row")
                        nc.scalar.activation(sd[:], var[:], AF.Sqrt,
                                             bias=eps_sb[:1, :])
                        rinv = pst.tile([1, NT], f32r, tag="row")
                        nc.vector.reciprocal(rinv[:], sd[:])
                        cc = pst.tile([1, NT], f32r, tag="row")
                        nc.vector.tensor_mul(cc[:], mu[:], rinv[:])
                        nc.vector.tensor_scalar_mul(cc[:], cc[:], -1.0)
                        pa = pps.tile([128, NT], f32, tag="mm")
                        mmr(pa[:], ones_sb[0:1, :], rinv[:], start=True, stop=True)
                        pc = pps.tile([128, NT], f32, tag="mm")
                        mmr(pc[:], ones_sb[0:1, :], cc[:], start=True, stop=True)
                        out = []
                        for k in range(KD):
                            if ln_ident:
                                y = px.tile([128, NT], f32r, tag="x")
                                nc.vector.tensor_mul(y[:], src[k][:], pa[:])
                                nc.vector.tensor_add(y[:], y[:], pc[:])
                            else:
                                t1 = plns.tile([128, NT], f32, tag="lnscr")
                                nc.vector.tensor_mul(t1[:], src[k][:], pa[:])
                                nc.vector.tensor_add(t1[:], t1[:], pc[:])
                                y = px.tile([128, NT], f32r, tag="x")
                                nc.vector.tensor_scalar(
                                    y[:], t1[:],
                                    pp_sb[:, s_col + k:s_col + k + 1],
                                    pp_sb[:, b_col + k:b_col + k + 1],
                                    OP.mult, OP.add)
                            out.append(y)
                        return out

                    y1t = layernorm(x2t, 0, 6)

                    # ---- FFN1 (+bias+relu, bf16 out) ----
                    ht = []
                    for half in range(2 if "ffn" not in ablate else 0):
                        w1h = []
                        for k in range(KD):
                            t = pwmed.tile([128, F // 2], f32r, tag="wmed")
                            nc.sync.dma_start(
                                t[:], d_w1[l, k * 128:(k + 1) * 128,
                                           half * (F // 2):(half + 1) * (F // 2)])
                            w1h.append(t)
                        for m in range(12):
                            fm = half * 12 + m
                            ps = pps.tile([128, NT], f32, tag="mm")
                            for k in range(KD):
                                mmr(ps[:], w1h[k][:, m * 128:(m + 1) * 128],
                                    y1t[k][:], start=(k == 0), stop=(k == KD - 1))
                            hb16 = ph.tile([128, NT], bf16, tag="h")
                            nc.scalar.activation(hb16[:], ps[:], AF.Relu,
                                                 bias=pp_sb[:, 42 + fm:43 + fm])
                            ht.append(hb16)

                    # ---- FFN2 (bf16) + bias + residual -> x3t ----
                    x3t = [] if "ffn" not in ablate else y1t
                    for half in range(2 if "ffn" not in ablate else 0):
                        w2h = []
                        for k in range(KF):
                            t = pw2.tile([128, 384], bf16, tag="w2")
                            nc.sync.dma_start(
                                t[:], d_w2[l, k * 128:(k + 1) * 128,
                                           half * 384:(half + 1) * 384])
                            w2h.append(t)
                        for m in range(3):
                            dm = half * 3 + m
                            ps = pps.tile([128, NT], f32, tag="mm")
                            for k in range(KF):
                                nc.tensor.matmul(ps[:],
                                                 w2h[k][:, m * 128:(m + 1) * 128],
                                                 ht[k][:], start=(k == 0),
                                                 stop=(k == KF - 1))
                            x3 = px.tile([128, NT], f32r, tag="x")
                            nc.vector.tensor_add(x3[:], ps[:], y1t[dm][:])
                            if b2_nz:
                                nc.vector.tensor_scalar_add(
                                    x3[:], x3[:], pp_sb[:, 24 + dm:25 + dm])
                            x3t.append(x3)

                    xt = layernorm(x3t, 12, 18)

                # ship final x (bf16) to DRAM for the AllGather
                for k in range(KD):
                    xb = plns.tile([128, NT], bf16, tag="xb16")
                    nc.vector.tensor_copy(xb[:], xt[k][:])
                    nc.sync.dma_start(ag_in[k * 128:(k + 1) * 128, :], xb[:])

            # ================= final: AllGather + vocab-sharded projection ======
            if not do_final:
                t_dump = pconst.tile([128, 8], bf16, tag="dump", name=f"dump{rep}")
                nc.sync.dma_start(t_dump[:], ag_in[0:128, 0:8])
                nc.sync.dma_start(d_out[0:128, 0:8], t_dump[:])
                return
            with ExitStack() as fin:
                if sim_mode:
                    for c in range(NCORE):
                        nc.sync.dma_start(ag_out[c], ag_in[:])
                else:
                    nc.gpsimd.collective_compute(
                        "AllGather", mybir.AluOpType.bypass,
                        replica_groups=[list(range(NCORE))],
                        ins=[ag_in.opt()], outs=[ag_out.opt()])

                pwo = fin.enter_context(tc.tile_pool(name=f"pwo{rep}", bufs=7))
                pxa = fin.enter_context(tc.tile_pool(name=f"pxa{rep}", bufs=14))
                pstrip = fin.enter_context(tc.tile_pool(name=f"pstrip{rep}", bufs=4))
                pstg = fin.enter_context(tc.tile_pool(name=f"pstg{rep}", bufs=6))
                pz = fin.enter_context(tc.tile_pool(name=f"pz{rep}", bufs=N_MT + 2))
                pzr = fin.enter_context(tc.tile_pool(name=f"pzr{rep}", bufs=6))
                pbo = (fin.enter_context(tc.tile_pool(name=f"pbo{rep}", bufs=1))
                       if bout_nz else None)

                wo_sb = []
                for k in range(KD):
                    t = pwo.tile([128, VC], bf16, tag="wo")
                    nc.sync.dma_start(t[:], d_wo[k * 128:(k + 1) * 128, :])
                    wo_sb.append(t)
                if bout_nz:
                    bo_sb = pbo.tile([128, VC], f32, tag="bo")
                    nc.sync.dma_start(bo_sb[:], d_bout[:])

                zg_sb = pzr.tile([128, N_MT], f32, tag="zg")
                nc.vector.memset(zg_sb[:], 1.0)
                rz_sb = pzr.tile([128, N_MT], f32, tag="rzf")

                strips = {}
                zp = {}
                for mt in range(N_MT):
                    c, j = mt // 4, mt % 4
                    m0, ml = MT_LOC[j]
                    xa = []
                    for k in range(KD):
                        t = pxa.tile([128, 128], bf16, tag="xa")
                        nc.sync.dma_start(t[:, :ml],
                                          ag_out[c, k * 128:(k + 1) * 128,
                                                 m0:m0 + ml])
                        xa.append(t)
                    strip = pstrip.tile([128, VC], f32, tag="strip")
                    strips[mt] = strip
                    zpt = pz.tile([128, len(VCH)], f32, tag="zp")
                    zp[mt] = zpt
                    for vi, (v0, vw) in enumerate(VCH):
                        ps = ppa.tile([128, 512], f32, tag="aux")
                        for k in range(KD):
                            nc.tensor.matmul(ps[:ml, :vw], xa[k][:, :ml],
                                             wo_sb[k][:, v0:v0 + vw],
                                             start=(k == 0), stop=(k == KD - 1))
                        if bout_nz:
                            nc.vector.tensor_add(ps[:ml, :vw], ps[:ml, :vw],
                                                 bo_sb[:ml, v0:v0 + vw])
                        nc.scalar.activation(strip[:ml, v0:v0 + vw], ps[:ml, :vw],
                                             AF.Exp,
                                             accum_out=zpt[:ml, vi:vi + 1])
                    nc.vector.tensor_reduce(zg_sb[:ml, mt:mt + 1], zpt[:ml, :],
                                            AX.X, OP.add)

                    if mt % ZGRP == ZGRP - 1:
                        g = mt // ZGRP
                        nc.sync.dma_start(z_in[g][:],
                                          zg_sb[:, g * ZGRP:(g + 1) * ZGRP])
                        if sim_mode:
                            nc.sync.dma_start(z_out[g][:], z_in[g][:])
                        else:
                            nc.gpsimd.collective_compute(
                                "AllReduce", mybir.AluOpType.add,
                                replica_groups=[list(range(NCORE))],
                                ins=[z_in[g].opt()], outs=[z_out[g].opt()])
                        zr = pzr.tile([128, ZGRP], f32, tag="zred")
                        nc.sync.dma_start(zr[:], z_out[g][:])
                        nc.vector.tensor_scalar_add(zr[:], zr[:], -float(N_VPAD))
                        nc.vector.reciprocal(
                            rz_sb[:, g * ZGRP:(g + 1) * ZGRP], zr[:])
                        for mt2 in range(g * ZGRP, (g + 1) * ZGRP):
                            c2, j2 = mt2 // 4, mt2 % 4
                            m02, ml2 = MT_LOC[j2]
                            r0 = c2 * NT + m02
                            for (v0, vw) in VCH:
                                so = pstg.tile([128, 512], bf16, tag="stg")
                                nc.vector.tensor_scalar_mul(
                                    so[:ml2, :vw],
                                    strips[mt2][:ml2, v0:v0 + vw],
                                    rz_sb[:ml2, mt2:mt2 + 1])
                                nc.sync.dma_start(d_out[r0:r0 + ml2, v0:v0 + vw],
                                                  so[:ml2, :vw])
                            del strips[mt2]

        for rep in range(repeat):
            _emit(rep)

    nc.compile()
    return nc


def _get_nc(flags, repeat=1, sim_mode=False, n_layers=L, do_final=True,
            ablate=()):
    key = (flags, repeat, sim_mode, n_layers, do_final, tuple(ablate))
    if key not in _CACHE:
        _CACHE[key] = _build(flags, repeat, sim_mode, n_layers, do_final,
                             ablate)
    return _CACHE[key]


def _prep(inputs):
    """Host-side preprocessing -> (per-core in_maps, specialization flags)."""
    x_img = np.asarray(inputs["image_token"], np.float32)
    tok = np.asarray(inputs["text_token"])
    tmask = np.asarray(inputs["text_mask"])
    temb = np.asarray(inputs["text_emb"], np.float32)
    semb = np.asarray(inputs["sep_emb"], np.float32)
    Wq = np.asarray(inputs["Wq"], np.float32)
    bq = np.asarray(inputs["bq"], np.float32)
    Wk = np.asarray(inputs["Wk"], np.float32)
    bk = np.asarray(inputs["bk"], np.float32)
    Wv = np.asarray(inputs["Wv"], np.float32)
    bv = np.asarray(inputs["bv"], np.float32)
    ln1_s = np.asarray(inputs["ln1_s"], np.float32)
    ln1_b = np.asarray(inputs["ln1_b"], np.float32)
    W1 = np.asarray(inputs["W1"], np.float32)
    b1 = np.asarray(inputs["b1"], np.float32)
    W2 = np.asarray(inputs["W2"], np.float32)
    b2 = np.asarray(inputs["b2"], np.float32)
    ln2_s = np.asarray(inputs["ln2_s"], np.float32)
    ln2_b = np.asarray(inputs["ln2_b"], np.float32)
    Wout = np.asarray(inputs["Wout"], np.float32)
    bout = np.asarray(inputs["bout"], np.float32)

    # x0 = [img | sep | emb[tokens]]
    x0 = np.concatenate(
        [x_img, np.broadcast_to(semb[None], (B, 1, D)), temb[tok]], axis=1)

    # maskT[b][t,s] = (t<=s) & combined[b,t]
    comb = np.concatenate(
        [np.ones((B, S - TXT), bool), tmask.astype(bool)], axis=1)
    tril_t = np.tril(np.ones((S, S), bool)).T  # [t,s]: t<=s
    maskt = (tril_t[None] & comb[:, :, None]).astype(ml_dtypes.bfloat16)

    # packed weights
    wqk = np.ascontiguousarray(np.concatenate([
        Wq.transpose(0, 2, 1, 3).reshape(L, D, D),
        Wk.transpose(0, 2, 1, 3).reshape(L, D, D)], axis=2))
    wv = np.ascontiguousarray(Wv.transpose(0, 2, 1, 3).reshape(L, D, D))
    w2 = W2.astype(ml_dtypes.bfloat16)

    # per-partition params: [L,128,72]
    pp = np.zeros((L, 128, 72), np.float32)

    def put(dst0, arr):  # arr [L, n*128]
        n = arr.shape[1] // 128
        pp[:, :, dst0:dst0 + n] = arr.reshape(L, n, 128).transpose(0, 2, 1)

    put(0, ln1_s); put(6, ln1_b); put(12, ln2_s); put(18, ln2_b); put(24, b2)
    put(30, np.concatenate([bq.reshape(L, D), bk.reshape(L, D)], axis=1))
    put(42, b1)

    wo_pad = np.zeros((D, V_PAD), ml_dtypes.bfloat16)
    wo_pad[:, :V] = Wout.astype(ml_dtypes.bfloat16)
    bout_pad = np.zeros(V_PAD, np.float32)
    bout_pad[:V] = bout

    ln_ident = bool(np.all(ln1_s == 1.0) and np.all(ln1_b == 0.0)
                    and np.all(ln2_s == 1.0) and np.all(ln2_b == 0.0))
    flags = (bool(np.any(bv)), bool(np.any(b2)), bool(np.any(bout)), ln_ident)

    ones = np.ones((128, 128), np.float32)
    ident = np.eye(128, dtype=np.float32)

    in_maps = []
    for c in range(NCORE):
        m = {
            "x0t": np.ascontiguousarray(
                x0[c * BC:(c + 1) * BC].reshape(NT, D).T),
            "wqk": wqk, "wv": wv, "w1": W1, "w2": w2,
            "wout": np.ascontiguousarray(wo_pad[:, c * VC:(c + 1) * VC]),
            "pp": pp,
            "maskt": np.ascontiguousarray(maskt[c * BC:(c + 1) * BC]),
            "ones": ones, "ident": ident,
        }
        if flags[0]:
            m["bvbc"] = np.ascontiguousarray(np.broadcast_to(
                bv.reshape(L, 1, D), (L, 128, D)))
        if flags[2]:
            m["boutbc"] = np.ascontiguousarray(np.broadcast_to(
                bout_pad[c * VC:(c + 1) * VC][None], (128, VC)))
        in_maps.append(m)
    return in_maps, flags


def kernel(**inputs):
    from concourse.bass_utils import run_bass_kernel_spmd
    in_maps, flags = _prep(inputs)
    nc = _get_nc(flags)
    res = run_bass_kernel_spmd(nc, in_maps, list(range(NCORE)))
    full = np.concatenate([res.results[c]["out"] for c in range(NCORE)], axis=1)
    return np.ascontiguousarray(
        full[:, :V].astype(np.float32).reshape(B, S, V))

